# revision 19
# baseline (speedup 1.0000x reference)
"""Contrastive loss kernel for Trainium2 (8 NeuronCores, batch-parallel).

Problem (hardcoded):
  X: (32, 16384, 256) f32   pair embeddings, e_a = X[..., :128], e_b = X[..., 128:]
  y: (32, 128, 128)  i32    adjacency in {0, 1}
  out: (32, 16384)   f32    where(y==1, dist2, relu(1 - dist2))

Sharding: data-parallel over batch, 4 batches per core, no communication.

Layout: the per-core (4, 16384, 256) X slice is treated as one flat stream of
65536 pairs; SBUF partition p owns the 512 consecutive pairs [p*512, (p+1)*512).
Each X DMA then moves one contiguous 32 KiB span per partition (128 descriptors
of 32 KiB per 4 MiB dma_start) instead of the 1 KiB-element gather the
pair%128 layout needs, and dist2 lands directly in output layout -- no
TensorE transpose, no PSUM.
"""

from contextlib import ExitStack

import numpy as np

import concourse.bass as bass
import concourse.tile as tile
from concourse import bacc, mybir
from concourse.bass_utils import run_bass_kernel_spmd

F32 = mybir.dt.float32
I32 = mybir.dt.int32

B, P, D = 32, 16384, 256
H = D // 2  # 128
ALPHA_MARGIN = 1.0
N_CORES = 8
BPC = B // N_CORES  # batches per core

PART = 128
G = BPC * P            # pairs per core (65536)
CPP = G // PART        # pairs per partition (512)


def tile_widths(w=32, taper=(16, 8, 4, 4), head=()):
    """Tile widths summing to CPP: steady stream of `w`, tapered tail so the
    post-DMA DVE drain is one small tile, not a full `w`-wide one; optional
    tapered head so compute engines start almost immediately."""
    tail = list(taper)
    hd = list(head)
    assert sum(tail) == w and all(w % t == 0 for t in tail)
    assert not hd or sum(hd) == w
    n = CPP // w - (2 if hd else 1)
    return hd + [w] * n + tail


def build_program(w=32, chunks=(128, 128, 128, 96, 32), nqueues=1, xbufs=4,
                  passes=1, taper=None, head=(), atw=16):
    """Per-core Bass program on the flat layout.

    Per full tile: DVE sub over the whole tile, then ACT computes dist2 for
    the first `atw` columns (per-column Square activation w/ accum) while DVE
    squares+reduces the remaining columns in two whole-block ops
    (stt square, tensor_reduce over the innermost axis).

    w: pairs per partition per DMA tile (tile = [128, w, 256] = w*32 KiB).
    chunks: output epilogue column counts (relu+predicate+store per chunk,
        overlapped with the X stream; small final chunk = short drain).
    nqueues: 1 = all X DMAs on the sync HWDGE ring; 2 = alternate sync/scalar.
    passes>1 repeats the whole computation (idempotent) -- used only for
    marginal-time benchmarking, never for the graded kernel."""
    if taper is None:
        taper = (16, 8, 4, 4) if w == 32 else (w,)
    widths = tile_widths(w, taper, head)
    if isinstance(chunks, int):
        chunks = (CPP // chunks,) * chunks
    assert sum(chunks) == CPP
    # chunk k covers columns [cbounds[k], cbounds[k+1])
    cbounds = [0]
    for c in chunks:
        cbounds.append(cbounds[-1] + c)

    nc = bacc.Bacc("TRN2", target_bir_lowering=False, debug=False,
                   num_devices=N_CORES)
    X = nc.dram_tensor("X", [G, D], F32, kind="ExternalInput").ap()
    Y = nc.dram_tensor("y", [G], I32, kind="ExternalInput").ap()
    O = nc.dram_tensor("out", [G], F32, kind="ExternalOutput").ap()

    Xv = X.rearrange("(p c) f -> p c f", c=CPP)
    Yv = Y.rearrange("(p c) -> p c", c=CPP)
    Ov = O.rearrange("(p c) -> p c", c=CPP)

    with tile.TileContext(nc) as tc, ExitStack() as ctx:
        xpool = ctx.enter_context(tc.tile_pool(name="x", bufs=xbufs))
        dpool = ctx.enter_context(tc.tile_pool(name="diff", bufs=2))
        rpool = ctx.enter_context(tc.tile_pool(name="res", bufs=2))
        spool = ctx.enter_context(tc.tile_pool(name="small", bufs=2))
        cpool = ctx.enter_context(tc.tile_pool(name="const", bufs=1))

        ones = cpool.tile([PART, 1], F32)
        nc.gpsimd.memset(ones[:], 1.0)

        for _ in range(passes):
            yt = spool.tile([PART, CPP], I32)
            nc.gpsimd.dma_start(yt[:], Yv)
            res = rpool.tile([PART, CPP], F32)
            outt = spool.tile([PART, CPP], F32)
            def epilogue(rc0, rtw):
                cend = rc0 + rtw
                for k in range(len(chunks)):
                    if not (rc0 < cbounds[k + 1] <= cend):
                        continue
                    k0, k1 = cbounds[k], cbounds[k + 1]
                    # relu(margin - d) in two DVE ops, keeping ACT pure-Square
                    # (a Relu activation would pay a ~1.3us table reload per
                    # func switch): min(d-1,0) = -relu(1-d), then negate.
                    nc.vector.tensor_scalar(
                        outt[:, k0:k1], res[:, k0:k1], ALPHA_MARGIN, 0.0,
                        mybir.AluOpType.subtract, mybir.AluOpType.min)
                    nc.vector.tensor_scalar(
                        outt[:, k0:k1], outt[:, k0:k1], -1.0, None,
                        mybir.AluOpType.mult)
                    nc.vector.copy_predicated(
                        outt[:, k0:k1], yt[:, k0:k1], res[:, k0:k1])
                    # out/y DMAs ride SWDGE (Pool): HWDGE DGE on the scalar
                    # ring stalls ACT's in-order SEQ for ~1.5us per DMA,
                    # starving the Square backlog at the tail.
                    nc.gpsimd.dma_start(Ov[:, k0:k1], outt[:, k0:k1])

            c0 = 0
            for t, tw in enumerate(widths):
                xt = xpool.tile([PART, tw, D], F32)
                dma_eng = nc.scalar if (nqueues == 2 and t % 2) else nc.sync
                dma_eng.dma_start(xt[:], Xv[:, c0:c0 + tw, :])
                dft = dpool.tile([PART, tw, H], F32)
                nc.vector.tensor_sub(dft[:], xt[:, :, 0:H], xt[:, :, H:D])
                # Split columns: ACT takes the first `a` as per-column
                # Square+accum (no feedback into DVE's in-order queue -- a
                # bulk ACT square with a DVE reduce behind it serializes the
                # tile scheduler's DVE stream on ACT); DVE squares+reduces
                # the rest in two whole-block ops.  Taper tiles stay all-DVE
                # so the post-DMA drain never waits on ACT's backlog.
                a = atw if tw == w else 0
                for j in range(a):
                    c = c0 + j
                    nc.scalar.activation(
                        dft[:, j, :], dft[:, j, :],
                        mybir.ActivationFunctionType.Square,
                        accum_out=res[:, c:c + 1],
                    )
                nc.vector.scalar_tensor_tensor(
                    out=dft[:, a:tw, :], in0=dft[:, a:tw, :], scalar=0.0,
                    in1=dft[:, a:tw, :],
                    op0=mybir.AluOpType.bypass, op1=mybir.AluOpType.mult)
                nc.vector.tensor_reduce(
                    res[:, c0 + a:c0 + tw], dft[:, a:tw, :],
                    axis=mybir.AxisListType.X, op=mybir.AluOpType.add)
                epilogue(c0, tw)
                c0 += tw

    nc.compile()
    return nc


_PROGRAM_CACHE = {}


def _get_program():
    if "nc" not in _PROGRAM_CACHE:
        _PROGRAM_CACHE["nc"] = build_program()
    return _PROGRAM_CACHE["nc"]


def kernel(X, y):
    import os
    if os.environ.get("BASS_TRACE"):
        # The axon NTFF trace path needs antenv.axon_hooks, which some
        # images lack; fall back to untraced execution rather than crash.
        try:
            import antenv.axon_hooks  # noqa: F401
        except ImportError:
            os.environ["BASS_NEVER_TRACE"] = "1"

    X = np.asarray(X, dtype=np.float32)
    y = np.asarray(y, dtype=np.int32).reshape(B, P)
    assert X.shape == (B, P, D)

    nc = _get_program()
    in_maps = [
        {"X": np.ascontiguousarray(X[c * BPC:(c + 1) * BPC]).reshape(G, D),
         "y": np.ascontiguousarray(y[c * BPC:(c + 1) * BPC]).reshape(G)}
        for c in range(N_CORES)
    ]
    # The axon-tunneled devices occasionally come up wedged from a prior
    # session (NRT_EXEC_UNIT_UNRECOVERABLE); a backend reset + retry clears it.
    last_err = None
    for attempt in range(3):
        try:
            res = run_bass_kernel_spmd(nc, in_maps, list(range(N_CORES)))
            break
        except Exception as e:  # transient device/tunnel failures
            last_err = e
            import time

            import jax
            try:
                jax.clear_caches()
            except Exception:
                pass
            try:
                jax._src.api.clear_backends()
            except Exception:
                pass
            time.sleep(5.0 * (attempt + 1))
    else:
        raise last_err
    out = np.concatenate(
        [res.results[c]["out"].reshape(BPC, P) for c in range(N_CORES)], axis=0)
    return out.astype(np.float32)


# revision 26
# speedup vs baseline: 1.4502x; 1.4502x over previous
"""Contrastive loss kernel for Trainium2 (8 NeuronCores, batch-parallel).

Problem (hardcoded):
  X: (32, 16384, 256) f32   pair embeddings, e_a = X[..., :128], e_b = X[..., 128:]
  y: (32, 128, 128)  i32    adjacency in {0, 1}
  out: (32, 16384)   f32    where(y==1, dist2, relu(1 - dist2))

Sharding: data-parallel over batch, 4 batches per core, no communication.

DMA layout (measured on HW, 8 cores contending for HBM):
  * pair%128 partition mapping: each X dma_start covers one CONTIGUOUS
    2 MiB HBM window (slots=16 pair-columns x 128 partitions x 1 KiB rows).
    Dense windows stream at ~354 GB/s/core (dmaonly marginal 190.7us/pass).
  * A flat per-partition-contiguous layout (32 KiB descriptors, 512 KiB
    stride between partitions) measured ~288 GB/s -- 128 strided streams
    schedule much worse in the HBM controller under 8-core contention.
    Descriptor size is NOT the lever; window density is.

Compute (decoupled from the X DMA ring so it can never stall it):
  * per tile: DVE sub -> ACT dist2 for first `atw` columns (per-column
    Square+accum), DVE stt-square + tensor_reduce(axis=X) for the rest.
  * per batch: PE transpose (pair=t*128+p -> out wants [t,p]), then a 3-op
    DVE epilogue (min/negate relu + copy_predicated) -- ACT stays
    pure-Square so it never reloads activation tables.
  * y/out DMAs ride SWDGE (Pool): a dependent DMA on an HWDGE ring parks
    that ring's sequencer on the dep semaphore, stalling every X DMA queued
    behind it (~8.5us/pass measured: full vs dmaonly).
"""

from contextlib import ExitStack

import numpy as np

import concourse.bass as bass
import concourse.tile as tile
from concourse import bacc, masks, mybir
from concourse.bass_utils import run_bass_kernel_spmd

F32 = mybir.dt.float32
BF16 = mybir.dt.bfloat16
I32 = mybir.dt.int32

B, P, D = 32, 16384, 256
H = D // 2  # 128
ALPHA_MARGIN = 1.0
N_CORES = 8
BPC = B // N_CORES  # batches per core

PART = 128
NCOLS = P // PART  # result columns per batch (128)


def build_program(slots=32, atw=20, xbufs=4, passes=1, dmaonly=False,
                  dma_split=False):
    """Per-core Bass program.  Shapes are per-core (full batch dim / 8).

    slots: pair-columns per X dma_start (tile = [128, slots, 256] =
        slots * 128 KiB window; 32 -> 4 MiB DMAs).
    atw: columns per tile computed on ACT (per-column Square+accum); rest go
        through DVE stt+tensor_reduce.  Default slots//2.
    passes>1 repeats the whole computation (idempotent) -- used only for
    marginal-time benchmarking, never for the graded kernel."""
    tiles = NCOLS // slots
    assert tiles * slots == NCOLS
    if atw is None:
        atw = slots // 2

    nc = bacc.Bacc("TRN2", target_bir_lowering=False, debug=False,
                   num_devices=N_CORES)
    X = nc.dram_tensor("X", [BPC, P, D], F32, kind="ExternalInput").ap()
    Y = nc.dram_tensor("y", [BPC, P], I32, kind="ExternalInput").ap()
    O = nc.dram_tensor("out", [BPC, P], F32, kind="ExternalOutput").ap()

    with tile.TileContext(nc) as tc, ExitStack() as ctx:
        xpool = ctx.enter_context(tc.tile_pool(name="x", bufs=xbufs))
        dpool = ctx.enter_context(tc.tile_pool(name="diff", bufs=3))
        rpool = ctx.enter_context(tc.tile_pool(name="res", bufs=2))
        ppool = ctx.enter_context(tc.tile_pool(name="psum", bufs=2,
                                               space="PSUM"))
        spool = ctx.enter_context(tc.tile_pool(name="small", bufs=2))
        opool = ctx.enter_context(tc.tile_pool(name="outb", bufs=2))
        cpool = ctx.enter_context(tc.tile_pool(name="const", bufs=1))

        ident = cpool.tile([PART, PART], F32)
        masks.make_identity(nc, ident[:])

        def epilogue(res, yt, b):
            outt = opool.tile([NCOLS, PART], F32)
            if dmaonly:
                nc.gpsimd.memset(outt[:], 0.0)
            else:
                # res[p, t] = dist2(pair t*128+p); transpose so partition = t
                pres = ppool.tile([NCOLS, PART], F32)
                nc.tensor.transpose(pres[:], res[:], ident[:])
                # relu(margin - d) without touching ACT's activation table:
                # min(d-1, 0) = -relu(1-d), then negate; then y==1 -> d.
                nc.vector.tensor_scalar(
                    outt[:], pres[:], ALPHA_MARGIN, 0.0,
                    mybir.AluOpType.subtract, mybir.AluOpType.min)
                nc.vector.tensor_scalar(
                    outt[:], outt[:], -1.0, None, mybir.AluOpType.mult)
                nc.vector.copy_predicated(outt[:], yt[:], pres[:])

            nc.gpsimd.dma_start(O[b].rearrange("(t p) -> t p", p=PART),
                                outt[:])

        pending = None  # batch whose epilogue hasn't been emitted yet
        for b in [b for _ in range(passes) for b in range(BPC)]:
            # pair index = t*128 + p  ->  [p, t, f] view of X[b]
            Xb = X[b].rearrange("(t p) f -> p t f", p=PART)
            res = rpool.tile([PART, NCOLS], F32)
            yt = spool.tile([NCOLS, PART], I32)
            nc.gpsimd.dma_start(yt[:], Y[b].rearrange("(t p) -> t p", p=PART))
            for g in range(tiles):
                xt = xpool.tile([PART, slots, D], F32)
                dma_eng = nc.scalar if (dma_split and g % 2) else nc.sync
                dma_eng.dma_start(xt[:], Xb[:, g * slots:(g + 1) * slots, :])
                if not dmaonly:
                    # bf16 diffs: the square+reduce work runs at 2x DVE/ACT
                    # throughput, dropping both engines to ~50% of the DMA
                    # period so compute jitter can never starve the X ring.
                    # dist2 accumulates in f32; measured rel err ~1e-3 vs the
                    # 2e-2 gate.
                    dft = dpool.tile([PART, slots, H], BF16)
                    nc.vector.tensor_sub(dft[:], xt[:, :, 0:H],
                                         xt[:, :, H:D])
                    c0 = g * slots
                    for j in range(atw):
                        nc.scalar.activation(
                            dft[:, j, :], dft[:, j, :],
                            mybir.ActivationFunctionType.Square,
                            accum_out=res[:, c0 + j:c0 + j + 1],
                        )
                    nc.vector.scalar_tensor_tensor(
                        out=dft[:, atw:slots, :], in0=dft[:, atw:slots, :],
                        scalar=0.0, in1=dft[:, atw:slots, :],
                        op0=mybir.AluOpType.bypass, op1=mybir.AluOpType.mult)
                    nc.vector.tensor_reduce(
                        res[:, c0 + atw:c0 + slots], dft[:, atw:slots, :],
                        axis=mybir.AxisListType.X, op=mybir.AluOpType.add)
                # software-pipeline: the previous batch's epilogue is emitted
                # inside this batch's tile stream, so its cross-engine waits
                # (ACT accum cols -> PE transpose -> DVE select) sit behind
                # this batch's subs in every engine queue and can never stall
                # the X DMA ring via xpool-slot starvation.
                if g == 1 and pending is not None:
                    epilogue(*pending)
                    pending = None
            pending = (res, yt, b)
        epilogue(*pending)

    nc.compile()
    return nc


_PROGRAM_CACHE = {}


def _get_program():
    if "nc" not in _PROGRAM_CACHE:
        _PROGRAM_CACHE["nc"] = build_program()
    return _PROGRAM_CACHE["nc"]


def kernel(X, y):
    import os
    if os.environ.get("BASS_TRACE"):
        # The axon NTFF trace path needs antenv.axon_hooks, which some
        # images lack; fall back to untraced execution rather than crash.
        try:
            import antenv.axon_hooks  # noqa: F401
        except ImportError:
            os.environ["BASS_NEVER_TRACE"] = "1"

    X = np.asarray(X, dtype=np.float32)
    y = np.asarray(y, dtype=np.int32).reshape(B, P)
    assert X.shape == (B, P, D)

    nc = _get_program()
    in_maps = [
        {"X": np.ascontiguousarray(X[c * BPC:(c + 1) * BPC]),
         "y": np.ascontiguousarray(y[c * BPC:(c + 1) * BPC])}
        for c in range(N_CORES)
    ]
    # The axon-tunneled devices occasionally come up wedged from a prior
    # session (NRT_EXEC_UNIT_UNRECOVERABLE); a backend reset + retry clears it.
    last_err = None
    for attempt in range(3):
        try:
            res = run_bass_kernel_spmd(nc, in_maps, list(range(N_CORES)))
            break
        except Exception as e:  # transient device/tunnel failures
            last_err = e
            import time

            import jax
            try:
                jax.clear_caches()
            except Exception:
                pass
            try:
                jax._src.api.clear_backends()
            except Exception:
                pass
            time.sleep(5.0 * (attempt + 1))
    else:
        raise last_err
    out = np.concatenate([res.results[c]["out"] for c in range(N_CORES)],
                         axis=0)
    return out.astype(np.float32)


# revision 29
# speedup vs baseline: 1.4829x; 1.0226x over previous
"""Contrastive loss kernel for Trainium2 (8 NeuronCores, batch-parallel).

Problem (hardcoded):
  X: (32, 16384, 256) f32   pair embeddings, e_a = X[..., :128], e_b = X[..., 128:]
  y: (32, 128, 128)  i32    adjacency in {0, 1}
  out: (32, 16384)   f32    where(y==1, dist2, relu(1 - dist2))

Sharding: data-parallel over batch, 4 batches per core, no communication.

DMA layout (measured on HW, 8 cores contending for HBM):
  * pair%128 partition mapping: each X dma_start covers one CONTIGUOUS
    2 MiB HBM window (slots=16 pair-columns x 128 partitions x 1 KiB rows).
    Dense windows stream at ~354 GB/s/core (dmaonly marginal 190.7us/pass).
  * A flat per-partition-contiguous layout (32 KiB descriptors, 512 KiB
    stride between partitions) measured ~288 GB/s -- 128 strided streams
    schedule much worse in the HBM controller under 8-core contention.
    Descriptor size is NOT the lever; window density is.

Compute (decoupled from the X DMA ring so it can never stall it):
  * per tile: DVE sub -> ACT dist2 for first `atw` columns (per-column
    Square+accum), DVE stt-square + tensor_reduce(axis=X) for the rest.
  * per batch: PE transpose (pair=t*128+p -> out wants [t,p]), then a 3-op
    DVE epilogue (min/negate relu + copy_predicated) -- ACT stays
    pure-Square so it never reloads activation tables.
  * y/out DMAs ride SWDGE (Pool): a dependent DMA on an HWDGE ring parks
    that ring's sequencer on the dep semaphore, stalling every X DMA queued
    behind it (~8.5us/pass measured: full vs dmaonly).
"""

from contextlib import ExitStack

import numpy as np

import concourse.bass as bass
import concourse.tile as tile
from concourse import bacc, masks, mybir
from concourse.bass_utils import run_bass_kernel_spmd

F32 = mybir.dt.float32
BF16 = mybir.dt.bfloat16
I32 = mybir.dt.int32

B, P, D = 32, 16384, 256
H = D // 2  # 128
ALPHA_MARGIN = 1.0
N_CORES = 8
BPC = B // N_CORES  # batches per core

PART = 128
NCOLS = P // PART  # result columns per batch (128)


def build_program(slots=32, atw=20, xbufs=5, passes=1, dmaonly=False,
                  dma_split=False, xcast=True):
    """Per-core Bass program.  Shapes are per-core (full batch dim / 8).

    slots: pair-columns per X dma_start (tile = [128, slots, 256] =
        slots * 128 KiB window; 32 -> 4 MiB DMAs).
    atw: columns per tile computed on ACT (per-column Square+accum); rest go
        through DVE stt+tensor_reduce.  Default slots//2.
    passes>1 repeats the whole computation (idempotent) -- used only for
    marginal-time benchmarking, never for the graded kernel."""
    tiles = NCOLS // slots
    assert tiles * slots == NCOLS
    if atw is None:
        atw = slots // 2

    nc = bacc.Bacc("TRN2", target_bir_lowering=False, debug=False,
                   num_devices=N_CORES)
    X = nc.dram_tensor("X", [BPC, P, D], F32, kind="ExternalInput").ap()
    Y = nc.dram_tensor("y", [BPC, P], I32, kind="ExternalInput").ap()
    O = nc.dram_tensor("out", [BPC, P], F32, kind="ExternalOutput").ap()

    with tile.TileContext(nc) as tc, ExitStack() as ctx:
        xpool = ctx.enter_context(tc.tile_pool(name="x", bufs=xbufs))
        dpool = ctx.enter_context(tc.tile_pool(name="diff", bufs=3))
        rpool = ctx.enter_context(tc.tile_pool(name="res", bufs=2))
        ppool = ctx.enter_context(tc.tile_pool(name="psum", bufs=2,
                                               space="PSUM"))
        spool = ctx.enter_context(tc.tile_pool(name="small", bufs=2))
        opool = ctx.enter_context(tc.tile_pool(name="outb", bufs=2))
        cpool = ctx.enter_context(tc.tile_pool(name="const", bufs=1))

        ident = cpool.tile([PART, PART], F32)
        masks.make_identity(nc, ident[:])

        def epilogue(res, yt, b):
            outt = opool.tile([NCOLS, PART], F32)
            if dmaonly:
                nc.gpsimd.memset(outt[:], 0.0)
            else:
                # res[p, t] = dist2(pair t*128+p); transpose so partition = t
                pres = ppool.tile([NCOLS, PART], F32)
                nc.tensor.transpose(pres[:], res[:], ident[:])
                # relu(margin - d) without touching ACT's activation table:
                # min(d-1, 0) = -relu(1-d), then negate; then y==1 -> d.
                nc.vector.tensor_scalar(
                    outt[:], pres[:], ALPHA_MARGIN, 0.0,
                    mybir.AluOpType.subtract, mybir.AluOpType.min)
                nc.vector.tensor_scalar(
                    outt[:], outt[:], -1.0, None, mybir.AluOpType.mult)
                nc.vector.copy_predicated(outt[:], yt[:], pres[:])

            small_eng.dma_start(O[b].rearrange("(t p) -> t p", p=PART),
                                outt[:])

        pending = None  # batch whose epilogue hasn't been emitted yet
        for b in [b for _ in range(passes) for b in range(BPC)]:
            # pair index = t*128 + p  ->  [p, t, f] view of X[b]
            Xb = X[b].rearrange("(t p) f -> p t f", p=PART)
            res = rpool.tile([PART, NCOLS], F32)
            yt = spool.tile([NCOLS, PART], I32)
            # xcast: X rides SWDGE with inline f32->bf16 cast (halves the sub
            # cost and xt footprint); y/out then move to the otherwise-empty
            # sync HWDGE ring where their dep-sem parking stalls nothing.
            small_eng = nc.sync if xcast else nc.gpsimd
            small_eng.dma_start(yt[:], Y[b].rearrange("(t p) -> t p", p=PART))
            for g in range(tiles):
                xt = xpool.tile([PART, slots, D], BF16 if xcast else F32)
                if xcast:
                    dma_eng = nc.gpsimd
                else:
                    dma_eng = nc.scalar if (dma_split and g % 2) else nc.sync
                dma_eng.dma_start(xt[:], Xb[:, g * slots:(g + 1) * slots, :])
                if not dmaonly:
                    # bf16 diffs: the square+reduce work runs at 2x DVE/ACT
                    # throughput, dropping both engines to ~50% of the DMA
                    # period so compute jitter can never starve the X ring.
                    # dist2 accumulates in f32; measured rel err ~1e-3 vs the
                    # 2e-2 gate.
                    dft = dpool.tile([PART, slots, H], BF16)
                    nc.vector.tensor_sub(dft[:], xt[:, :, 0:H],
                                         xt[:, :, H:D])
                    c0 = g * slots
                    for j in range(atw):
                        nc.scalar.activation(
                            dft[:, j, :], dft[:, j, :],
                            mybir.ActivationFunctionType.Square,
                            accum_out=res[:, c0 + j:c0 + j + 1],
                        )
                    nc.vector.scalar_tensor_tensor(
                        out=dft[:, atw:slots, :], in0=dft[:, atw:slots, :],
                        scalar=0.0, in1=dft[:, atw:slots, :],
                        op0=mybir.AluOpType.bypass, op1=mybir.AluOpType.mult)
                    nc.vector.tensor_reduce(
                        res[:, c0 + atw:c0 + slots], dft[:, atw:slots, :],
                        axis=mybir.AxisListType.X, op=mybir.AluOpType.add)
                # software-pipeline: the previous batch's epilogue is emitted
                # inside this batch's tile stream, so its cross-engine waits
                # (ACT accum cols -> PE transpose -> DVE select) sit behind
                # this batch's subs in every engine queue and can never stall
                # the X DMA ring via xpool-slot starvation.
                if g == 1 and pending is not None:
                    epilogue(*pending)
                    pending = None
            pending = (res, yt, b)
        epilogue(*pending)

    nc.compile()
    return nc


_PROGRAM_CACHE = {}


def _get_program():
    if "nc" not in _PROGRAM_CACHE:
        _PROGRAM_CACHE["nc"] = build_program()
    return _PROGRAM_CACHE["nc"]


def kernel(X, y):
    import os
    if os.environ.get("BASS_TRACE"):
        # The axon NTFF trace path needs antenv.axon_hooks, which some
        # images lack; fall back to untraced execution rather than crash.
        try:
            import antenv.axon_hooks  # noqa: F401
        except ImportError:
            os.environ["BASS_NEVER_TRACE"] = "1"

    X = np.asarray(X, dtype=np.float32)
    y = np.asarray(y, dtype=np.int32).reshape(B, P)
    assert X.shape == (B, P, D)

    nc = _get_program()
    in_maps = [
        {"X": np.ascontiguousarray(X[c * BPC:(c + 1) * BPC]),
         "y": np.ascontiguousarray(y[c * BPC:(c + 1) * BPC])}
        for c in range(N_CORES)
    ]
    # The axon-tunneled devices occasionally come up wedged from a prior
    # session (NRT_EXEC_UNIT_UNRECOVERABLE); a backend reset + retry clears it.
    last_err = None
    for attempt in range(3):
        try:
            res = run_bass_kernel_spmd(nc, in_maps, list(range(N_CORES)))
            break
        except Exception as e:  # transient device/tunnel failures
            last_err = e
            import time

            import jax
            try:
                jax.clear_caches()
            except Exception:
                pass
            try:
                jax._src.api.clear_backends()
            except Exception:
                pass
            time.sleep(5.0 * (attempt + 1))
    else:
        raise last_err
    out = np.concatenate([res.results[c]["out"] for c in range(N_CORES)],
                         axis=0)
    return out.astype(np.float32)
